# revision 14
# baseline (speedup 1.0000x reference)
"""Self-contained Trainium2 Bass kernel for a 2-layer GCN encoder (8 cores).

reference semantics (PyG GCNConv x2):
    out = Ahat @ relu(Ahat @ x @ W1 + b1) @ W2 + b2
    Ahat = D^-1/2 (A + I) D^-1/2,  deg = dst-counts + self-loops.

Strategy (graph/node parallel over 8 NeuronCores):
  * aggregation is linear => reorder matmuls around it:
        layer1: h = relu( dinv * (agg(dinv*x) @ W1) + b1 )
        layer2: out = dinv * agg( (dinv*h) @ W2 ) + b2
    so layer-2 gathers 128-wide rows instead of 256-wide.
  * nodes are degree-sorted into 128-node tiles dealt round-robin to the
    8 cores; every core runs one identical program (SPMD) over its own
    edge data.
  * neighbor rows are fetched with batched InstDMAGatherAnt calls of
    <=1024 rows (HW limit) rotating over the 4 SWDGE queues.
  * gathered rows land at list position (row j -> partition j%128,
    block j//128); a per-call selection matrix (built on DVE from
    precomputed dst-partition codes vs an iota tile) routes rows to
    their destination node partitions in an accumulating PE matmul.
    Pad rows carry code 255 -> zero selection column -> contribute 0.
  * layer-1 dst tiles are processed chunk-major (4 chunks of ~24 slots);
    as each chunk's h'@W2 rows finish, a per-chunk AllGather ships them,
    overlapping the collective with the remaining layer-1 work.  Layer-2
    gathers read the per-chunk replicated tables.
  * per-(tile,split) segment lengths are equalized across cores so the
    instruction stream is core-independent; per-core idx/codes arrays
    carry the data differences.  Trailing pad indices are -1 (skipped by
    HW, no bandwidth); mid-list pads gather row 0 and are masked by
    code 255.
"""

import os
import sys
import numpy as np

for _p in ("/opt/trn_rl_repo",):
    if _p not in sys.path and os.path.isdir(_p):
        sys.path.insert(0, _p)

P = 128
NQ = 4          # xs table quarters (int16 index range)
NCHUNK = 4      # cc chunks (collective pipelining)


class Cfg:
    def __init__(self, N=100000, E=3200000, F_IN=256, F_HID=256, F_OUT=128,
                 C=8, gtiles=4, nbcap=8, scratch=65536, gbufs=12):
        self.N, self.E = N, E
        self.F_IN, self.F_HID, self.F_OUT = F_IN, F_HID, F_OUT
        self.C = C
        self.NBCAP = nbcap          # max 128-row blocks per dma_gather
        self.SCRATCH = scratch      # SWDGE ring bytes per partition
        self.GBUFS = gbufs
        assert nbcap * P <= scratch // 16
        nt = (N + P) // P
        nt = ((nt + C - 1) // C) * C
        self.TPC = nt // C
        self.NT = nt
        self.NPAD = nt * P
        self.NPADL = self.TPC * P
        self.QS = self.NPAD // NQ
        self.GT = gtiles
        assert self.NPAD > self.N and self.NPAD % NQ == 0
        assert self.QS <= 32768
        # cc chunks: split TPC slots into NCHUNK consecutive ranges
        base = self.TPC // NCHUNK
        sizes = [base] * NCHUNK
        sizes[-1] += self.TPC - base * NCHUNK
        self.CH_SIZES = sizes                    # slots per chunk
        self.CH_OFF = np.concatenate([[0], np.cumsum(sizes)]).astype(int)
        assert max(s * C * P for s in sizes) <= 32768
        # chunk-major groups of consecutive slots
        self.groups = []
        self.chunk_last_group = {}
        for k in range(NCHUNK):
            lo, hi = self.CH_OFF[k], self.CH_OFF[k + 1]
            s = lo
            while s < hi:
                e = min(s + gtiles, hi)
                self.groups.append(list(range(s, e)))
                s = e
            self.chunk_last_group[k] = len(self.groups) - 1
        self.NG = len(self.groups)


def _schedule(cfg, groups, split_of_edge, srel_of_edge, ecore, eslot, epart,
              nsplit, m_base, col_base):
    """Build a call schedule for one layer.

    split_of_edge: which table-piece each edge's source lives in
    srel_of_edge:  int16 row index within that piece
    Returns (calls, idx_cat, codes, m_tot, tot_cols16).
    """
    C, TPC, NBCAP = cfg.C, cfg.TPC, cfg.NBCAP
    key = (ecore * TPC + eslot) * nsplit + split_of_edge
    ordk = np.argsort(key, kind="stable")
    srel_s = srel_of_edge[ordk]
    epart_s = epart[ordk]
    L = np.bincount(key, minlength=C * TPC * nsplit).reshape(C, TPC, nsplit)
    off = np.zeros(C * TPC * nsplit + 1, np.int64)
    np.cumsum(L.reshape(-1), out=off[1:])
    Lmax = L.max(axis=0)  # [TPC, nsplit]

    calls = []
    first_of = {}
    last_of = {}
    tot = 0
    m_tot = 0
    for g, slots in enumerate(groups):
        for q in range(nsplit):
            A = {}
            a = 0
            for s in slots:
                A[s] = a
                a += int(Lmax[s, q])
            total_len = a
            nb_all = (total_len + P - 1) // P
            b0 = 0
            while b0 < nb_all:
                nb = min(NBCAP, nb_all - b0)
                if nb <= 0:
                    break
                descs = []
                for b in range(b0, b0 + nb):
                    lo, hi = b * P, (b + 1) * P
                    for s in slots:
                        if A[s] < hi and A[s] + int(Lmax[s, q]) > lo:
                            di = (len(calls), len(descs))
                            if s not in first_of:
                                first_of[s] = di
                            last_of[s] = di
                            descs.append([b - b0, s, m_base + m_tot,
                                          False, False])
                            m_tot += 1
                calls.append(dict(q=q, N=nb * P, col0=col_base + tot // 16,
                                  A=A, b0=b0, descs=descs, group=g))
                tot += nb * P
                b0 += nb
    for s, (ci, di) in first_of.items():
        calls[ci]["descs"][di][3] = True
    for s, (ci, di) in last_of.items():
        calls[ci]["descs"][di][4] = True
    glast = {}
    for ci, call in enumerate(calls):
        call["epilogue"] = False
        glast[call["group"]] = ci
    for ci in glast.values():
        calls[ci]["epilogue"] = True

    # per-core idx + codes
    idx_cat = np.full((C, tot), -1, np.int16)
    codes = np.full((C, P, m_tot), 255, np.uint8)
    gq_cache = {}
    for call in calls:
        q = call["q"]
        g = call["group"]
        base = (call["col0"] - col_base) * 16
        ncall = call["N"]
        w0 = call["b0"] * P
        if (g, q) not in gq_cache:
            a_tot = sum(int(Lmax[s, q]) for s in groups[g])
            slotid = np.full(a_tot, -1, np.int64)
            idx_full = np.zeros((C, a_tot), np.int16)
            dstp_full = np.full((C, a_tot), 255, np.uint8)
            for s, a in call["A"].items():
                slotid[a:a + int(Lmax[s, q])] = s
                for c in range(C):
                    k = (c * TPC + s) * nsplit + q
                    n = int(L[c, s, q])
                    seg = slice(off[k], off[k] + n)
                    idx_full[c, a:a + n] = srel_s[seg]
                    dstp_full[c, a:a + n] = epart_s[seg]
            gq_cache[(g, q)] = (slotid, idx_full, dstp_full, a_tot)
        slotid, idx_full, dstp_full, a_tot = gq_cache[(g, q)]
        w1 = min(w0 + ncall, a_tot)
        n_real = w1 - w0
        idx_cat[:, base:base + n_real] = idx_full[:, w0:w1]
        for c in range(C):
            dstp = np.full(ncall, 255, np.uint8)
            dstp[:n_real] = dstp_full[c, w0:w1]
            sid = np.full(ncall, -1, np.int64)
            sid[:n_real] = slotid[w0:w1]
            for b, s, m, _, _ in call["descs"]:
                rows = slice(b * P, (b + 1) * P)
                codes[c, :, m - m_base] = np.where(sid[rows] == s,
                                                   dstp[rows], 255)
    return calls, idx_cat, codes, m_tot, tot // 16


def _prep(cfg, x, edge_index):
    import ml_dtypes
    N, C, TPC = cfg.N, cfg.C, cfg.TPC
    src = np.asarray(edge_index[0], dtype=np.int64)
    dst = np.asarray(edge_index[1], dtype=np.int64)
    deg = np.bincount(dst, minlength=N).astype(np.int64) + 1
    dinv = (1.0 / np.sqrt(deg)).astype(np.float32)

    order = np.argsort(-deg, kind="stable")
    i = np.arange(N)
    g_tile = i // P
    lane = g_tile % C
    srow = g_tile // C
    core_of = np.empty(N, np.int64)
    slot_of = np.empty(N, np.int64)
    part_of = np.empty(N, np.int64)
    core_of[order] = np.where(srow % 2 == 0, lane, C - 1 - lane)
    slot_of[order] = srow
    part_of[order] = i % P
    pad_id = (core_of * cfg.NPADL + slot_of * P + part_of).astype(np.int64)

    # self-loops are NOT gathered: they are one dense row-block per slot,
    # added in the epilogues (xs row for L1, cc_in row for L2).  Keeping
    # them in the gather lists put every (core,slot)'s 128 self-edges in
    # one table split, inflating the cross-core Lmax padding by ~9%.
    s_all = src
    d_all = dst

    ecore = core_of[d_all]
    eslot = slot_of[d_all]
    epart = part_of[d_all].astype(np.uint8)

    # layer 1: xs table split into NQ quarters by padded position
    spos = pad_id[s_all]
    equar = spos // cfg.QS
    srel1 = (spos % cfg.QS).astype(np.int16)

    # layer 2: cc table split into NCHUNK chunks by source SLOT range
    scor = core_of[s_all]
    sslt = slot_of[s_all]
    spart = part_of[s_all]
    chunk_of_slot = np.searchsorted(cfg.CH_OFF[1:], sslt, side="right")
    n_k = np.asarray(cfg.CH_SIZES)[chunk_of_slot]
    o_k = cfg.CH_OFF[chunk_of_slot]
    srel2_full = scor * n_k * P + (sslt - o_k) * P + spart
    assert srel2_full.max() < 32768
    srel2 = srel2_full.astype(np.int16)

    calls1, idx1, codes1, m1, cols1 = _schedule(
        cfg, cfg.groups, equar, srel1, ecore, eslot, epart, NQ, 0, 0)
    calls2, idx2, codes2, m2, cols2 = _schedule(
        cfg, cfg.groups, chunk_of_slot, srel2, ecore, eslot, epart,
        NCHUNK, m1, cols1)

    idx_cat = np.concatenate([idx1, idx2], axis=1)
    codes = np.concatenate([codes1, codes2], axis=2)
    cols = cols1 + cols2
    idx_w = idx_cat.reshape(C, cols, 16).transpose(0, 2, 1)
    idx_tabs = np.tile(idx_w, (1, 8, 1)).copy()          # [C,128,cols]
    codes_bf = codes.astype(ml_dtypes.bfloat16)

    dinv_pad = np.zeros(cfg.NPAD, np.float32)
    dinv_pad[pad_id] = dinv
    dinv_tabs = dinv_pad.reshape(C, TPC, P).transpose(0, 2, 1).copy()

    xs_f32 = np.zeros((cfg.NPAD, cfg.F_IN), np.float32)
    xs_f32[pad_id] = np.asarray(x, np.float32) * dinv[:, None]
    xs_pad = xs_f32.astype(ml_dtypes.bfloat16)
    xs_own = xs_f32.reshape(C, cfg.NPADL, cfg.F_IN)

    return dict(calls1=calls1, calls2=calls2, M=m1 + m2, COLS=cols,
                idx_tabs=idx_tabs, codes=codes_bf, dinv_tabs=dinv_tabs,
                xs_pad=xs_pad, xs_own=xs_own, core_of=core_of,
                slot_of=slot_of, part_of=part_of)


def _build(cfg, prep):
    import concourse.bass as bass
    import concourse.bacc as bacc
    import concourse.mybir as mybir
    import concourse.tile as tile
    from concourse.library_config import mlp

    f32 = mybir.dt.float32
    bf16 = mybir.dt.bfloat16
    i16 = mybir.dt.int16
    TPC, QS = cfg.TPC, cfg.QS
    F, FH, FO = cfg.F_IN, cfg.F_HID, cfg.F_OUT
    calls1, calls2, M, COLS = (prep["calls1"], prep["calls2"], prep["M"],
                               prep["COLS"])
    NB = cfg.NBCAP

    nc = bacc.Bacc("TRN2", target_bir_lowering=False, debug=False,
                   enable_asserts=False, num_devices=cfg.C,
                   num_swdge_queues=4,
                   dynamic_dma_scratch_size=cfg.SCRATCH)

    xs_t = nc.dram_tensor("xs", [cfg.NPAD, F], bf16, kind="ExternalInput")
    xso_t = nc.dram_tensor("xso", [cfg.NPADL, F], f32, kind="ExternalInput")
    idx_t = nc.dram_tensor("idx", [P, COLS], i16, kind="ExternalInput")
    codes_t = nc.dram_tensor("codes", [P, M], bf16, kind="ExternalInput")
    dinv_t = nc.dram_tensor("dinv", [P, TPC], f32, kind="ExternalInput")
    w1_t = nc.dram_tensor("w1", [F, FH], f32, kind="ExternalInput")
    b1_t = nc.dram_tensor("b1r", [P, FH], f32, kind="ExternalInput")
    w2_t = nc.dram_tensor("w2", [FH, FO], f32, kind="ExternalInput")
    b2_t = nc.dram_tensor("b2r", [P, FO], f32, kind="ExternalInput")
    ident_t = nc.dram_tensor("identf", [P, P], f32, kind="ExternalInput")
    iota_t = nc.dram_tensor("iota", [P, P], f32, kind="ExternalInput")
    out_t = nc.dram_tensor("out", [cfg.NPADL, FO], f32, kind="ExternalOutput")
    cc_in = nc.dram_tensor("cc_in", [cfg.NPADL, FO], bf16)
    cc_ch = [nc.dram_tensor(f"cc_ch{k}", [cfg.C * cfg.CH_SIZES[k] * P, FO],
                            bf16, addr_space="Shared")
             for k in range(NCHUNK)]

    with tile.TileContext(nc) as tc:
        with (
            tc.tile_pool(name="persist", bufs=1) as pp,
            tc.tile_pool(name="g", bufs=cfg.GBUFS) as gp,
            tc.tile_pool(name="sp", bufs=6) as spool,
            tc.tile_pool(name="ix", bufs=4) as ixp,
            tc.tile_pool(name="ep", bufs=3) as ep,
            tc.tile_pool(name="psA", bufs=5, space="PSUM") as psA,
            tc.tile_pool(name="psT", bufs=1, space="PSUM") as psT,
            tc.tile_pool(name="psB", bufs=2, space="PSUM") as psB,
        ):
            nc.gpsimd.load_library(mlp)
            codes_sb = pp.tile([P, M], bf16)
            nc.sync.dma_start(out=codes_sb[:], in_=codes_t[:, :])
            dinv_all = pp.tile([P, TPC], f32)
            nc.sync.dma_start(out=dinv_all[:], in_=dinv_t[:, :])
            ident = pp.tile([P, P], f32)
            nc.sync.dma_start(out=ident[:], in_=ident_t[:, :])
            iota_f = pp.tile([P, P], f32)
            nc.sync.dma_start(out=iota_f[:], in_=iota_t[:, :])
            iota_b = pp.tile([P, P], bf16, tag="iotab")
            nc.vector.tensor_copy(iota_b[:], iota_f[:])
            w_sb = {}
            for nm, wt, fo in (("w1", w1_t, FH), ("w2", w2_t, FO)):
                lst = []
                for k in range(2):
                    w = pp.tile([P, fo], f32, tag=f"{nm}_{k}")
                    nc.sync.dma_start(out=w[:], in_=wt[k * P:(k + 1) * P, :])
                    lst.append(w)
                w_sb[nm] = lst
            b1_sb = pp.tile([P, FH], f32, tag="b1")
            nc.sync.dma_start(out=b1_sb[:], in_=b1_t[:, :])
            b2_sb = pp.tile([P, FO], f32, tag="b2")
            nc.sync.dma_start(out=b2_sb[:], in_=b2_t[:, :])

            for _ in range(cfg.GBUFS):
                gz = gp.tile([P, NB, F], bf16, tag="g")
                nc.vector.memset(gz[:], 0.0)

            def mm_T(psum_out, src_sb, wl):
                nk = src_sb.shape[1] // P
                for k2 in range(nk):
                    psum_tt = psT.tile([P, P], f32, tag="tt")
                    nc.tensor.transpose(psum_tt[:],
                                        src_sb[:, k2 * P:(k2 + 1) * P],
                                        ident[:])
                    sT = ep.tile([P, P], f32, tag="sT")
                    nc.scalar.copy(sT[:], psum_tt[:])
                    nc.tensor.matmul(psum_out[:], lhsT=sT[:], rhs=wl[k2][:],
                                     start=(k2 == 0), stop=(k2 == nk - 1))

            qn = [0]

            def layer(calls, table_of, Fw, first):
                psums = {}
                # batched idx loads: one DMA per (group, q), emitted
                # in-stream right before that (g, q)'s first gather call
                gq_sizes = {}
                for call in calls:
                    k = (call["group"], call["q"])
                    if k not in gq_sizes:
                        gq_sizes[k] = [call["col0"], 0]
                    gq_sizes[k][1] += call["N"] // 16
                cur_ix = [None, None]
                for call in calls:
                    q = call["q"]
                    gq = (call["group"], q)
                    if cur_ix[0] != gq:
                        c0, ctot = gq_sizes[gq]
                        ix = ixp.tile([P, ctot], i16, tag="ix",
                                      padded_shape=[P, (NB * P // 16) * 8])
                        nc.scalar.dma_start(out=ix[:],
                                            in_=idx_t[:, c0:c0 + ctot])
                        cur_ix = [gq, ix]
                    ix = cur_ix[1]
                    o = call["col0"] - gq_sizes[gq][0]
                    nb = call["N"] // P
                    g = gp.tile([P, nb, Fw], bf16, tag="g",
                                padded_shape=[P, NB * (F // Fw), Fw])
                    nc.gpsimd.dma_gather(
                        g[:], table_of(q),
                        ix[:, o:o + call["N"] // 16],
                        call["N"], call["N"], Fw,
                        queue_num=qn[0] % 4)
                    qn[0] += 1
                    descs = call["descs"]
                    nw = len(descs)
                    m0 = descs[0][2]
                    S = spool.tile([P, nw, P], bf16, tag="S",
                                   padded_shape=[P, NB + 6, P])
                    nc.vector.tensor_tensor(
                        out=S[:],
                        in0=codes_sb[:, m0:m0 + nw].unsqueeze(2)
                            .to_broadcast([P, nw, P]),
                        in1=iota_b[:].unsqueeze(1)
                            .to_broadcast([P, nw, P]),
                        op=mybir.AluOpType.is_equal)
                    for j, (b, s, m, st, sp_) in enumerate(descs):
                        if st:
                            psums[s] = psA.tile([P, Fw], f32, tag="agg",
                                                padded_shape=[P, F],
                                                name=f"ps{s}")
                        nc.tensor.matmul(psums[s][:], lhsT=S[:, j, :],
                                         rhs=g[:, b, :], start=st,
                                         stop=sp_)
                    if call["epilogue"]:
                        gslots = cfg.groups[call["group"]]
                        for s in gslots:
                            psum_agg = psums.pop(s, None)
                            if first:
                                # self-loop term: agg += dinv_d * x_d
                                xso = ep.tile([P, F], f32, tag="xso")
                                nc.scalar.dma_start(
                                    out=xso[:],
                                    in_=xso_t[s * P:(s + 1) * P, :])
                                agg_s = ep.tile([P, F], f32, tag="aggs")
                                if psum_agg is None:
                                    nc.scalar.copy(agg_s[:], xso[:])
                                else:
                                    nc.vector.tensor_add(agg_s[:],
                                                         psum_agg[:],
                                                         xso[:])
                                psum_h = psB.tile([P, FH], f32, tag="h")
                                mm_T(psum_h, agg_s, w_sb["w1"])
                                t1 = ep.tile([P, FH], f32, tag="t1")
                                nc.vector.tensor_scalar_mul(
                                    t1[:], psum_h[:], dinv_all[:, s:s + 1])
                                t2 = ep.tile([P, FH], f32, tag="t2")
                                nc.vector.tensor_add(t2[:], t1[:], b1_sb[:])
                                hs = ep.tile([P, FH], f32, tag="hs")
                                nc.scalar.activation(
                                    hs[:], t2[:],
                                    mybir.ActivationFunctionType.Relu,
                                    scale=dinv_all[:, s:s + 1])
                                psum_o = psB.tile([P, FO], f32, tag="h",
                                                  padded_shape=[P, FH])
                                mm_T(psum_o, hs, w_sb["w2"])
                                os_ = ep.tile([P, FO], bf16, tag="os")
                                nc.vector.tensor_copy(os_[:], psum_o[:])
                                nc.sync.dma_start(
                                    out=cc_in[s * P:(s + 1) * P, :],
                                    in_=os_[:])
                            else:
                                # self-loop term: agg += dinv_d * z_d
                                zso_b = ep.tile([P, FO], bf16, tag="zsob")
                                nc.scalar.dma_start(
                                    out=zso_b[:],
                                    in_=cc_in[s * P:(s + 1) * P, :])
                                u0 = ep.tile([P, FO], f32, tag="u0")
                                if psum_agg is None:
                                    nc.scalar.copy(u0[:], zso_b[:])
                                else:
                                    nc.vector.tensor_add(u0[:],
                                                         psum_agg[:],
                                                         zso_b[:])
                                u1 = ep.tile([P, FO], f32, tag="u1")
                                nc.vector.tensor_scalar_mul(
                                    u1[:], u0[:], dinv_all[:, s:s + 1])
                                u2 = ep.tile([P, FO], f32, tag="u2")
                                nc.vector.tensor_add(u2[:], u1[:], b2_sb[:])
                                nc.sync.dma_start(
                                    out=out_t[s * P:(s + 1) * P, :],
                                    in_=u2[:])
                        if first:
                            for k in range(NCHUNK):
                                if cfg.chunk_last_group[k] == call["group"]:
                                    lo = cfg.CH_OFF[k] * P
                                    hi = cfg.CH_OFF[k + 1] * P
                                    nc.gpsimd.collective_compute(
                                        "AllGather", mybir.AluOpType.bypass,
                                        replica_groups=[list(range(cfg.C))],
                                        ins=[cc_in[lo:hi, :].opt()],
                                        outs=[cc_ch[k].ap().opt()],
                                    )

            mode = os.environ.get("GCN_MODE", "full")
            layer(calls1, lambda q: xs_t[q * QS:(q + 1) * QS, :], F,
                  first=True)
            if mode == "full":
                layer(calls2, lambda k: cc_ch[k][:, :], FO, first=False)

    nc.compile()
    return nc, None


def _run(cfg, nc, prep, W1, b1, W2, b2, trace=False):
    from concourse.bass_utils import run_bass_kernel_spmd
    b1r = np.broadcast_to(np.asarray(b1, np.float32), (P, cfg.F_HID)).copy()
    b2r = np.broadcast_to(np.asarray(b2, np.float32), (P, cfg.F_OUT)).copy()
    iota = np.tile(np.arange(P, dtype=np.float32), (P, 1))
    in_maps = []
    for c in range(cfg.C):
        in_maps.append({
            "xs": prep["xs_pad"],
            "xso": prep["xs_own"][c],
            "idx": prep["idx_tabs"][c],
            "codes": prep["codes"][c],
            "dinv": prep["dinv_tabs"][c],
            "w1": np.asarray(W1, np.float32),
            "b1r": b1r,
            "w2": np.asarray(W2, np.float32),
            "b2r": b2r,
            "identf": np.eye(P, dtype=np.float32),
            "iota": iota,
        })
    res = run_bass_kernel_spmd(nc, in_maps, list(range(cfg.C)), trace=trace)
    outs = np.stack([res.results[c]["out"] for c in range(cfg.C)])
    out_full = np.empty((cfg.N, cfg.F_OUT), np.float32)
    co, so, po = prep["core_of"], prep["slot_of"], prep["part_of"]
    out_full[:] = outs[co, so * P + po]
    return out_full, res


def kernel(x, edge_index, W1, b1, W2, b2):
    cfg = Cfg()
    prep = _prep(cfg, x, edge_index)
    nc, _ = _build(cfg, prep)
    out, _ = _run(cfg, nc, prep, W1, b1, W2, b2,
                  trace=bool(int(os.environ.get("GCN_TRACE", "0"))))
    return out


# revision 15
# speedup vs baseline: 1.2929x; 1.2929x over previous
"""Self-contained Trainium2 Bass kernel for a 2-layer GCN encoder (8 cores).

reference semantics (PyG GCNConv x2):
    out = Ahat @ relu(Ahat @ x @ W1 + b1) @ W2 + b2
    Ahat = D^-1/2 (A + I) D^-1/2,  deg = dst-counts + self-loops.

Strategy (graph/node parallel over 8 NeuronCores):
  * aggregation is linear => reorder matmuls around it:
        layer1: h = relu( dinv * (agg(dinv*x) @ W1) + b1 )
        layer2: out = dinv * agg( (dinv*h) @ W2 ) + b2
    so layer-2 gathers 128-wide rows instead of 256-wide.
  * nodes are degree-sorted into 128-node tiles dealt round-robin to the
    8 cores; every core runs one identical program (SPMD) over its own
    edge data.
  * neighbor rows are fetched with batched InstDMAGatherAnt calls of
    <=1024 rows (HW limit) rotating over the 4 SWDGE queues.
  * gathered rows land at list position (row j -> partition j%128,
    block j//128); a per-call selection matrix (built on DVE from
    precomputed dst-partition codes vs an iota tile) routes rows to
    their destination node partitions in an accumulating PE matmul.
    Pad rows carry code 255 -> zero selection column -> contribute 0.
  * layer-1 dst tiles are processed chunk-major (4 chunks of ~24 slots);
    as each chunk's h'@W2 rows finish, a per-chunk AllGather ships them,
    overlapping the collective with the remaining layer-1 work.  Layer-2
    gathers read the per-chunk replicated tables.
  * per-(tile,split) segment lengths are equalized across cores so the
    instruction stream is core-independent; per-core idx/codes arrays
    carry the data differences.  Trailing pad indices are -1 (skipped by
    HW, no bandwidth); mid-list pads gather row 0 and are masked by
    code 255.
"""

import os
import sys
import numpy as np

for _p in ("/opt/trn_rl_repo",):
    if _p not in sys.path and os.path.isdir(_p):
        sys.path.insert(0, _p)

P = 128
NQ = 4          # xs table quarters (int16 index range)
NCHUNK = 4      # cc chunks (collective pipelining)


class Cfg:
    def __init__(self, N=100000, E=3200000, F_IN=256, F_HID=256, F_OUT=128,
                 C=8, gtiles=4, nbcap=8, scratch=65536, gbufs=12):
        self.N, self.E = N, E
        self.F_IN, self.F_HID, self.F_OUT = F_IN, F_HID, F_OUT
        self.C = C
        self.NBCAP = nbcap          # max 128-row blocks per dma_gather
        self.SCRATCH = scratch      # SWDGE ring bytes per partition
        self.GBUFS = gbufs
        assert nbcap * P <= scratch // 16
        nt = (N + P) // P
        nt = ((nt + C - 1) // C) * C
        self.TPC = nt // C
        self.NT = nt
        self.NPAD = nt * P
        self.NPADL = self.TPC * P
        self.QS = self.NPAD // NQ
        self.GT = gtiles
        assert self.NPAD > self.N and self.NPAD % NQ == 0
        assert self.QS <= 32768
        # cc chunks: split TPC slots into NCHUNK consecutive ranges
        base = self.TPC // NCHUNK
        sizes = [base] * NCHUNK
        sizes[-1] += self.TPC - base * NCHUNK
        self.CH_SIZES = sizes                    # slots per chunk
        self.CH_OFF = np.concatenate([[0], np.cumsum(sizes)]).astype(int)
        assert max(s * C * P for s in sizes) <= 32768
        # chunk-major groups of consecutive slots
        self.groups = []
        self.chunk_last_group = {}
        for k in range(NCHUNK):
            lo, hi = self.CH_OFF[k], self.CH_OFF[k + 1]
            s = lo
            while s < hi:
                e = min(s + gtiles, hi)
                self.groups.append(list(range(s, e)))
                s = e
            self.chunk_last_group[k] = len(self.groups) - 1
        self.NG = len(self.groups)


def _schedule(cfg, groups, split_of_edge, srel_of_edge, ecore, eslot, epart,
              nsplit, m_base, col_base):
    """Build a call schedule for one layer.

    split_of_edge: which table-piece each edge's source lives in
    srel_of_edge:  int16 row index within that piece
    Returns (calls, idx_cat, codes, m_tot, tot_cols16).
    """
    C, TPC, NBCAP = cfg.C, cfg.TPC, cfg.NBCAP
    key = (ecore * TPC + eslot) * nsplit + split_of_edge
    ordk = np.argsort(key, kind="stable")
    srel_s = srel_of_edge[ordk]
    epart_s = epart[ordk]
    L = np.bincount(key, minlength=C * TPC * nsplit).reshape(C, TPC, nsplit)
    off = np.zeros(C * TPC * nsplit + 1, np.int64)
    np.cumsum(L.reshape(-1), out=off[1:])
    Lmax = L.max(axis=0)  # [TPC, nsplit]

    calls = []
    first_of = {}
    last_of = {}
    tot = 0
    m_tot = 0
    for g, slots in enumerate(groups):
        for q in range(nsplit):
            A = {}
            a = 0
            for s in slots:
                A[s] = a
                a += int(Lmax[s, q])
            total_len = a
            nb_all = (total_len + P - 1) // P
            b0 = 0
            while b0 < nb_all:
                nb = min(NBCAP, nb_all - b0)
                if nb <= 0:
                    break
                descs = []
                for b in range(b0, b0 + nb):
                    lo, hi = b * P, (b + 1) * P
                    for s in slots:
                        if A[s] < hi and A[s] + int(Lmax[s, q]) > lo:
                            di = (len(calls), len(descs))
                            if s not in first_of:
                                first_of[s] = di
                            last_of[s] = di
                            descs.append([b - b0, s, m_base + m_tot,
                                          False, False])
                            m_tot += 1
                calls.append(dict(q=q, N=nb * P, col0=col_base + tot // 16,
                                  A=A, b0=b0, descs=descs, group=g))
                tot += nb * P
                b0 += nb
    for s, (ci, di) in first_of.items():
        calls[ci]["descs"][di][3] = True
    for s, (ci, di) in last_of.items():
        calls[ci]["descs"][di][4] = True
    glast = {}
    for ci, call in enumerate(calls):
        call["epilogue"] = False
        glast[call["group"]] = ci
    for ci in glast.values():
        calls[ci]["epilogue"] = True

    # per-core idx + codes
    idx_cat = np.full((C, tot), -1, np.int16)
    codes = np.full((C, P, m_tot), 255, np.uint8)
    gq_cache = {}
    for call in calls:
        q = call["q"]
        g = call["group"]
        base = (call["col0"] - col_base) * 16
        ncall = call["N"]
        w0 = call["b0"] * P
        if (g, q) not in gq_cache:
            a_tot = sum(int(Lmax[s, q]) for s in groups[g])
            slotid = np.full(a_tot, -1, np.int64)
            idx_full = np.zeros((C, a_tot), np.int16)
            dstp_full = np.full((C, a_tot), 255, np.uint8)
            for s, a in call["A"].items():
                slotid[a:a + int(Lmax[s, q])] = s
                for c in range(C):
                    k = (c * TPC + s) * nsplit + q
                    n = int(L[c, s, q])
                    seg = slice(off[k], off[k] + n)
                    idx_full[c, a:a + n] = srel_s[seg]
                    dstp_full[c, a:a + n] = epart_s[seg]
            gq_cache[(g, q)] = (slotid, idx_full, dstp_full, a_tot)
        slotid, idx_full, dstp_full, a_tot = gq_cache[(g, q)]
        w1 = min(w0 + ncall, a_tot)
        n_real = w1 - w0
        idx_cat[:, base:base + n_real] = idx_full[:, w0:w1]
        for c in range(C):
            dstp = np.full(ncall, 255, np.uint8)
            dstp[:n_real] = dstp_full[c, w0:w1]
            sid = np.full(ncall, -1, np.int64)
            sid[:n_real] = slotid[w0:w1]
            for b, s, m, _, _ in call["descs"]:
                rows = slice(b * P, (b + 1) * P)
                codes[c, :, m - m_base] = np.where(sid[rows] == s,
                                                   dstp[rows], 255)
    return calls, idx_cat, codes, m_tot, tot // 16


def _prep(cfg, x, edge_index):
    import ml_dtypes
    N, C, TPC = cfg.N, cfg.C, cfg.TPC
    src = np.asarray(edge_index[0], dtype=np.int64)
    dst = np.asarray(edge_index[1], dtype=np.int64)
    deg = np.bincount(dst, minlength=N).astype(np.int64) + 1
    dinv = (1.0 / np.sqrt(deg)).astype(np.float32)

    order = np.argsort(-deg, kind="stable")
    i = np.arange(N)
    g_tile = i // P
    lane = g_tile % C
    srow = g_tile // C
    core_of = np.empty(N, np.int64)
    slot_of = np.empty(N, np.int64)
    part_of = np.empty(N, np.int64)
    core_of[order] = np.where(srow % 2 == 0, lane, C - 1 - lane)
    slot_of[order] = srow
    part_of[order] = i % P
    pad_id = (core_of * cfg.NPADL + slot_of * P + part_of).astype(np.int64)

    # self-loops are NOT gathered: they are one dense row-block per slot,
    # added in the epilogues (xs row for L1, cc_in row for L2).  Keeping
    # them in the gather lists put every (core,slot)'s 128 self-edges in
    # one table split, inflating the cross-core Lmax padding by ~9%.
    s_all = src
    d_all = dst

    ecore = core_of[d_all]
    eslot = slot_of[d_all]
    epart = part_of[d_all].astype(np.uint8)

    # layer 1: xs table split into NQ quarters by padded position
    spos = pad_id[s_all]
    equar = spos // cfg.QS
    srel1 = (spos % cfg.QS).astype(np.int16)

    # layer 2: cc table split into NCHUNK chunks by source SLOT range
    scor = core_of[s_all]
    sslt = slot_of[s_all]
    spart = part_of[s_all]
    chunk_of_slot = np.searchsorted(cfg.CH_OFF[1:], sslt, side="right")
    n_k = np.asarray(cfg.CH_SIZES)[chunk_of_slot]
    o_k = cfg.CH_OFF[chunk_of_slot]
    srel2_full = scor * n_k * P + (sslt - o_k) * P + spart
    assert srel2_full.max() < 32768
    srel2 = srel2_full.astype(np.int16)

    calls1, idx1, codes1, m1, cols1 = _schedule(
        cfg, cfg.groups, equar, srel1, ecore, eslot, epart, NQ, 0, 0)
    calls2, idx2, codes2, m2, cols2 = _schedule(
        cfg, cfg.groups, chunk_of_slot, srel2, ecore, eslot, epart,
        NCHUNK, m1, cols1)

    idx_cat = np.concatenate([idx1, idx2], axis=1)
    codes = np.concatenate([codes1, codes2], axis=2)
    cols = cols1 + cols2
    idx_w = idx_cat.reshape(C, cols, 16).transpose(0, 2, 1)
    idx_tabs = np.tile(idx_w, (1, 8, 1)).copy()          # [C,128,cols]
    codes_bf = codes.astype(ml_dtypes.bfloat16)

    dinv_pad = np.zeros(cfg.NPAD, np.float32)
    dinv_pad[pad_id] = dinv
    dinv_tabs = dinv_pad.reshape(C, TPC, P).transpose(0, 2, 1).copy()

    xs_f32 = np.zeros((cfg.NPAD, cfg.F_IN), np.float32)
    xs_f32[pad_id] = np.asarray(x, np.float32) * dinv[:, None]
    xs_pad = xs_f32.astype(ml_dtypes.bfloat16)
    xs_own = xs_f32.reshape(C, cfg.NPADL, cfg.F_IN)

    return dict(calls1=calls1, calls2=calls2, M=m1 + m2, COLS=cols,
                idx_tabs=idx_tabs, codes=codes_bf, dinv_tabs=dinv_tabs,
                xs_pad=xs_pad, xs_own=xs_own, core_of=core_of,
                slot_of=slot_of, part_of=part_of)


def _build(cfg, prep):
    import concourse.bass as bass
    import concourse.bacc as bacc
    import concourse.mybir as mybir
    import concourse.tile as tile
    from concourse.library_config import mlp

    f32 = mybir.dt.float32
    bf16 = mybir.dt.bfloat16
    i16 = mybir.dt.int16
    TPC, QS = cfg.TPC, cfg.QS
    F, FH, FO = cfg.F_IN, cfg.F_HID, cfg.F_OUT
    calls1, calls2, M, COLS = (prep["calls1"], prep["calls2"], prep["M"],
                               prep["COLS"])
    NB = cfg.NBCAP

    nc = bacc.Bacc("TRN2", target_bir_lowering=False, debug=False,
                   enable_asserts=False, num_devices=cfg.C,
                   num_swdge_queues=4,
                   dynamic_dma_scratch_size=cfg.SCRATCH)

    xs_t = nc.dram_tensor("xs", [cfg.NPAD, F], bf16, kind="ExternalInput")
    xso_t = nc.dram_tensor("xso", [cfg.NPADL, F], f32, kind="ExternalInput")
    idx_t = nc.dram_tensor("idx", [P, COLS], i16, kind="ExternalInput")
    codes_t = nc.dram_tensor("codes", [P, M], bf16, kind="ExternalInput")
    dinv_t = nc.dram_tensor("dinv", [P, TPC], f32, kind="ExternalInput")
    w1_t = nc.dram_tensor("w1", [F, FH], f32, kind="ExternalInput")
    b1_t = nc.dram_tensor("b1r", [P, FH], f32, kind="ExternalInput")
    w2_t = nc.dram_tensor("w2", [FH, FO], f32, kind="ExternalInput")
    b2_t = nc.dram_tensor("b2r", [P, FO], f32, kind="ExternalInput")
    ident_t = nc.dram_tensor("identf", [P, P], f32, kind="ExternalInput")
    iota_t = nc.dram_tensor("iota", [P, P], f32, kind="ExternalInput")
    out_t = nc.dram_tensor("out", [cfg.NPADL, FO], f32, kind="ExternalOutput")
    cc_in = nc.dram_tensor("cc_in", [cfg.NPADL, FO], bf16)
    cc_ch = [nc.dram_tensor(f"cc_ch{k}", [cfg.C * cfg.CH_SIZES[k] * P, FO],
                            bf16, addr_space="Shared")
             for k in range(NCHUNK)]

    with tile.TileContext(nc) as tc:
        with (
            tc.tile_pool(name="persist", bufs=1) as pp,
            tc.tile_pool(name="g", bufs=cfg.GBUFS) as gp,
            tc.tile_pool(name="sp", bufs=6) as spool,
            tc.tile_pool(name="ix", bufs=4) as ixp,
            tc.tile_pool(name="ep", bufs=3) as ep,
            tc.tile_pool(name="slp", bufs=8) as slp,
            tc.tile_pool(name="psA", bufs=5, space="PSUM") as psA,
            tc.tile_pool(name="psT", bufs=1, space="PSUM") as psT,
            tc.tile_pool(name="psB", bufs=2, space="PSUM") as psB,
        ):
            nc.gpsimd.load_library(mlp)
            codes_sb = pp.tile([P, M], bf16)
            nc.sync.dma_start(out=codes_sb[:], in_=codes_t[:, :])
            dinv_all = pp.tile([P, TPC], f32)
            nc.sync.dma_start(out=dinv_all[:], in_=dinv_t[:, :])
            ident = pp.tile([P, P], f32)
            nc.sync.dma_start(out=ident[:], in_=ident_t[:, :])
            iota_f = pp.tile([P, P], f32)
            nc.sync.dma_start(out=iota_f[:], in_=iota_t[:, :])
            iota_b = pp.tile([P, P], bf16, tag="iotab")
            nc.vector.tensor_copy(iota_b[:], iota_f[:])
            w_sb = {}
            for nm, wt, fo in (("w1", w1_t, FH), ("w2", w2_t, FO)):
                lst = []
                for k in range(2):
                    w = pp.tile([P, fo], f32, tag=f"{nm}_{k}")
                    nc.sync.dma_start(out=w[:], in_=wt[k * P:(k + 1) * P, :])
                    lst.append(w)
                w_sb[nm] = lst
            b1_sb = pp.tile([P, FH], f32, tag="b1")
            nc.sync.dma_start(out=b1_sb[:], in_=b1_t[:, :])
            b2_sb = pp.tile([P, FO], f32, tag="b2")
            nc.sync.dma_start(out=b2_sb[:], in_=b2_t[:, :])

            for _ in range(cfg.GBUFS):
                gz = gp.tile([P, NB, F], bf16, tag="g")
                nc.vector.memset(gz[:], 0.0)

            def mm_T(psum_out, src_sb, wl):
                nk = src_sb.shape[1] // P
                for k2 in range(nk):
                    psum_tt = psT.tile([P, P], f32, tag="tt")
                    nc.tensor.transpose(psum_tt[:],
                                        src_sb[:, k2 * P:(k2 + 1) * P],
                                        ident[:])
                    sT = ep.tile([P, P], f32, tag="sT")
                    nc.scalar.copy(sT[:], psum_tt[:])
                    nc.tensor.matmul(psum_out[:], lhsT=sT[:], rhs=wl[k2][:],
                                     start=(k2 == 0), stop=(k2 == nk - 1))

            qn = [0]

            def layer(calls, table_of, Fw, first):
                psums = {}
                # batched idx loads: one DMA per (group, q), emitted
                # in-stream right before that (g, q)'s first gather call
                gq_sizes = {}
                for call in calls:
                    k = (call["group"], call["q"])
                    if k not in gq_sizes:
                        gq_sizes[k] = [call["col0"], 0]
                    gq_sizes[k][1] += call["N"] // 16
                cur_ix = [None, None]
                for call in calls:
                    q = call["q"]
                    gq = (call["group"], q)
                    if cur_ix[0] != gq:
                        c0, ctot = gq_sizes[gq]
                        ix = ixp.tile([P, ctot], i16, tag="ix",
                                      padded_shape=[P, (NB * P // 16) * 8])
                        nc.scalar.dma_start(out=ix[:],
                                            in_=idx_t[:, c0:c0 + ctot])
                        cur_ix = [gq, ix]
                    ix = cur_ix[1]
                    o = call["col0"] - gq_sizes[gq][0]
                    nb = call["N"] // P
                    g = gp.tile([P, nb, Fw], bf16, tag="g",
                                padded_shape=[P, NB * (F // Fw), Fw])
                    nc.gpsimd.dma_gather(
                        g[:], table_of(q),
                        ix[:, o:o + call["N"] // 16],
                        call["N"], call["N"], Fw,
                        queue_num=qn[0] % 4)
                    qn[0] += 1
                    descs = call["descs"]
                    nw = len(descs)
                    m0 = descs[0][2]
                    S = spool.tile([P, nw, P], bf16, tag="S",
                                   padded_shape=[P, NB + 6, P])
                    nc.vector.tensor_tensor(
                        out=S[:],
                        in0=codes_sb[:, m0:m0 + nw].unsqueeze(2)
                            .to_broadcast([P, nw, P]),
                        in1=iota_b[:].unsqueeze(1)
                            .to_broadcast([P, nw, P]),
                        op=mybir.AluOpType.is_equal)
                    for j, (b, s, m, st, sp_) in enumerate(descs):
                        if st:
                            psums[s] = psA.tile([P, Fw], f32, tag="agg",
                                                padded_shape=[P, F],
                                                name=f"ps{s}")
                        nc.tensor.matmul(psums[s][:], lhsT=S[:, j, :],
                                         rhs=g[:, b, :], start=st,
                                         stop=sp_)
                    if call["epilogue"]:
                        gslots = cfg.groups[call["group"]]
                        selft = {}
                        for s in gslots:
                            # self-loop term rows, prefetched for the
                            # whole group so DMA latencies overlap
                            if first:
                                xso = slp.tile([P, F], f32, tag="xso")
                                nc.scalar.dma_start(
                                    out=xso[:],
                                    in_=xso_t[s * P:(s + 1) * P, :])
                            else:
                                xso = slp.tile([P, FO], bf16, tag="zsob")
                                nc.scalar.dma_start(
                                    out=xso[:],
                                    in_=cc_in[s * P:(s + 1) * P, :])
                            selft[s] = xso
                        for s in gslots:
                            psum_agg = psums.pop(s, None)
                            if first:
                                xso = selft[s]
                                agg_s = ep.tile([P, F], f32, tag="aggs")
                                if psum_agg is None:
                                    nc.scalar.copy(agg_s[:], xso[:])
                                else:
                                    nc.vector.tensor_add(agg_s[:],
                                                         psum_agg[:],
                                                         xso[:])
                                psum_h = psB.tile([P, FH], f32, tag="h")
                                mm_T(psum_h, agg_s, w_sb["w1"])
                                t1 = ep.tile([P, FH], f32, tag="t1")
                                nc.vector.tensor_scalar_mul(
                                    t1[:], psum_h[:], dinv_all[:, s:s + 1])
                                t2 = ep.tile([P, FH], f32, tag="t2")
                                nc.vector.tensor_add(t2[:], t1[:], b1_sb[:])
                                hs = ep.tile([P, FH], f32, tag="hs")
                                nc.scalar.activation(
                                    hs[:], t2[:],
                                    mybir.ActivationFunctionType.Relu,
                                    scale=dinv_all[:, s:s + 1])
                                psum_o = psB.tile([P, FO], f32, tag="h",
                                                  padded_shape=[P, FH])
                                mm_T(psum_o, hs, w_sb["w2"])
                                os_ = ep.tile([P, FO], bf16, tag="os")
                                nc.vector.tensor_copy(os_[:], psum_o[:])
                                nc.sync.dma_start(
                                    out=cc_in[s * P:(s + 1) * P, :],
                                    in_=os_[:])
                            else:
                                zso_b = selft[s]
                                u0 = ep.tile([P, FO], f32, tag="u0")
                                if psum_agg is None:
                                    nc.scalar.copy(u0[:], zso_b[:])
                                else:
                                    nc.vector.tensor_add(u0[:],
                                                         psum_agg[:],
                                                         zso_b[:])
                                u1 = ep.tile([P, FO], f32, tag="u1")
                                nc.vector.tensor_scalar_mul(
                                    u1[:], u0[:], dinv_all[:, s:s + 1])
                                u2 = ep.tile([P, FO], f32, tag="u2")
                                nc.vector.tensor_add(u2[:], u1[:], b2_sb[:])
                                nc.sync.dma_start(
                                    out=out_t[s * P:(s + 1) * P, :],
                                    in_=u2[:])
                        if first:
                            for k in range(NCHUNK):
                                if cfg.chunk_last_group[k] == call["group"]:
                                    lo = cfg.CH_OFF[k] * P
                                    hi = cfg.CH_OFF[k + 1] * P
                                    nc.gpsimd.collective_compute(
                                        "AllGather", mybir.AluOpType.bypass,
                                        replica_groups=[list(range(cfg.C))],
                                        ins=[cc_in[lo:hi, :].opt()],
                                        outs=[cc_ch[k].ap().opt()],
                                    )

            mode = os.environ.get("GCN_MODE", "full")
            layer(calls1, lambda q: xs_t[q * QS:(q + 1) * QS, :], F,
                  first=True)
            if mode == "full":
                layer(calls2, lambda k: cc_ch[k][:, :], FO, first=False)

    nc.compile()
    return nc, None


def _run(cfg, nc, prep, W1, b1, W2, b2, trace=False):
    from concourse.bass_utils import run_bass_kernel_spmd
    b1r = np.broadcast_to(np.asarray(b1, np.float32), (P, cfg.F_HID)).copy()
    b2r = np.broadcast_to(np.asarray(b2, np.float32), (P, cfg.F_OUT)).copy()
    iota = np.tile(np.arange(P, dtype=np.float32), (P, 1))
    in_maps = []
    for c in range(cfg.C):
        in_maps.append({
            "xs": prep["xs_pad"],
            "xso": prep["xs_own"][c],
            "idx": prep["idx_tabs"][c],
            "codes": prep["codes"][c],
            "dinv": prep["dinv_tabs"][c],
            "w1": np.asarray(W1, np.float32),
            "b1r": b1r,
            "w2": np.asarray(W2, np.float32),
            "b2r": b2r,
            "identf": np.eye(P, dtype=np.float32),
            "iota": iota,
        })
    res = run_bass_kernel_spmd(nc, in_maps, list(range(cfg.C)), trace=trace)
    outs = np.stack([res.results[c]["out"] for c in range(cfg.C)])
    out_full = np.empty((cfg.N, cfg.F_OUT), np.float32)
    co, so, po = prep["core_of"], prep["slot_of"], prep["part_of"]
    out_full[:] = outs[co, so * P + po]
    return out_full, res


def kernel(x, edge_index, W1, b1, W2, b2):
    cfg = Cfg()
    prep = _prep(cfg, x, edge_index)
    nc, _ = _build(cfg, prep)
    out, _ = _run(cfg, nc, prep, W1, b1, W2, b2,
                  trace=bool(int(os.environ.get("GCN_TRACE", "0"))))
    return out


# revision 16
# speedup vs baseline: 1.3011x; 1.0064x over previous
"""Self-contained Trainium2 Bass kernel for a 2-layer GCN encoder (8 cores).

reference semantics (PyG GCNConv x2):
    out = Ahat @ relu(Ahat @ x @ W1 + b1) @ W2 + b2
    Ahat = D^-1/2 (A + I) D^-1/2,  deg = dst-counts + self-loops.

Strategy (graph/node parallel over 8 NeuronCores):
  * aggregation is linear => reorder matmuls around it:
        layer1: h = relu( dinv * (agg(dinv*x) @ W1) + b1 )
        layer2: out = dinv * agg( (dinv*h) @ W2 ) + b2
    so layer-2 gathers 128-wide rows instead of 256-wide.
  * nodes are degree-sorted into 128-node tiles dealt round-robin to the
    8 cores; every core runs one identical program (SPMD) over its own
    edge data.
  * neighbor rows are fetched with batched InstDMAGatherAnt calls of
    <=1024 rows (HW limit) rotating over the 4 SWDGE queues.
  * gathered rows land at list position (row j -> partition j%128,
    block j//128); a per-call selection matrix (built on DVE from
    precomputed dst-partition codes vs an iota tile) routes rows to
    their destination node partitions in an accumulating PE matmul.
    Pad rows carry code 255 -> zero selection column -> contribute 0.
  * layer-1 dst tiles are processed chunk-major (4 chunks of ~24 slots);
    as each chunk's h'@W2 rows finish, a per-chunk AllGather ships them,
    overlapping the collective with the remaining layer-1 work.  Layer-2
    gathers read the per-chunk replicated tables.
  * per-(tile,split) segment lengths are equalized across cores so the
    instruction stream is core-independent; per-core idx/codes arrays
    carry the data differences.  Trailing pad indices are -1 (skipped by
    HW, no bandwidth); mid-list pads gather row 0 and are masked by
    code 255.
"""

import os
import sys
import numpy as np

for _p in ("/opt/trn_rl_repo",):
    if _p not in sys.path and os.path.isdir(_p):
        sys.path.insert(0, _p)

P = 128
NQ = 4          # xs table quarters (int16 index range)
NCHUNK = 4      # cc chunks (collective pipelining)


class Cfg:
    def __init__(self, N=100000, E=3200000, F_IN=256, F_HID=256, F_OUT=128,
                 C=8, gtiles=4, nbcap=8, scratch=65536, gbufs=12):
        self.N, self.E = N, E
        self.F_IN, self.F_HID, self.F_OUT = F_IN, F_HID, F_OUT
        self.C = C
        self.NBCAP = nbcap          # max 128-row blocks per dma_gather
        self.SCRATCH = scratch      # SWDGE ring bytes per partition
        self.GBUFS = gbufs
        assert nbcap * P <= scratch // 16
        nt = (N + P) // P
        nt = ((nt + C - 1) // C) * C
        self.TPC = nt // C
        self.NT = nt
        self.NPAD = nt * P
        self.NPADL = self.TPC * P
        self.QS = self.NPAD // NQ
        self.GT = gtiles
        assert self.NPAD > self.N and self.NPAD % NQ == 0
        assert self.QS <= 32768
        # cc chunks: split TPC slots into NCHUNK consecutive ranges.
        # The last chunk is small so its AllGather (which gates the tail
        # of every layer-2 group) finishes quickly after layer 1 ends.
        sizes = [30, 30, 28, 10]
        assert sum(sizes) == self.TPC and len(sizes) == NCHUNK
        self.CH_SIZES = sizes                    # slots per chunk
        self.CH_OFF = np.concatenate([[0], np.cumsum(sizes)]).astype(int)
        assert max(s * C * P for s in sizes) <= 32768
        # chunk-major groups of consecutive slots
        self.groups = []
        self.chunk_last_group = {}
        for k in range(NCHUNK):
            lo, hi = self.CH_OFF[k], self.CH_OFF[k + 1]
            s = lo
            while s < hi:
                e = min(s + gtiles, hi)
                self.groups.append(list(range(s, e)))
                s = e
            self.chunk_last_group[k] = len(self.groups) - 1
        self.NG = len(self.groups)


def _schedule(cfg, groups, split_of_edge, srel_of_edge, ecore, eslot, epart,
              nsplit, m_base, col_base):
    """Build a call schedule for one layer.

    split_of_edge: which table-piece each edge's source lives in
    srel_of_edge:  int16 row index within that piece
    Returns (calls, idx_cat, codes, m_tot, tot_cols16).
    """
    C, TPC, NBCAP = cfg.C, cfg.TPC, cfg.NBCAP
    key = (ecore * TPC + eslot) * nsplit + split_of_edge
    ordk = np.argsort(key, kind="stable")
    srel_s = srel_of_edge[ordk]
    epart_s = epart[ordk]
    L = np.bincount(key, minlength=C * TPC * nsplit).reshape(C, TPC, nsplit)
    off = np.zeros(C * TPC * nsplit + 1, np.int64)
    np.cumsum(L.reshape(-1), out=off[1:])
    Lmax = L.max(axis=0)  # [TPC, nsplit]

    calls = []
    first_of = {}
    last_of = {}
    tot = 0
    m_tot = 0
    for g, slots in enumerate(groups):
        for q in range(nsplit):
            A = {}
            a = 0
            for s in slots:
                A[s] = a
                a += int(Lmax[s, q])
            total_len = a
            nb_all = (total_len + P - 1) // P
            b0 = 0
            while b0 < nb_all:
                nb = min(NBCAP, nb_all - b0)
                if nb <= 0:
                    break
                descs = []
                for b in range(b0, b0 + nb):
                    lo, hi = b * P, (b + 1) * P
                    for s in slots:
                        if A[s] < hi and A[s] + int(Lmax[s, q]) > lo:
                            di = (len(calls), len(descs))
                            if s not in first_of:
                                first_of[s] = di
                            last_of[s] = di
                            descs.append([b - b0, s, m_base + m_tot,
                                          False, False])
                            m_tot += 1
                calls.append(dict(q=q, N=nb * P, col0=col_base + tot // 16,
                                  A=A, b0=b0, descs=descs, group=g))
                tot += nb * P
                b0 += nb
    for s, (ci, di) in first_of.items():
        calls[ci]["descs"][di][3] = True
    for s, (ci, di) in last_of.items():
        calls[ci]["descs"][di][4] = True
    glast = {}
    for ci, call in enumerate(calls):
        call["epilogue"] = False
        glast[call["group"]] = ci
    for ci in glast.values():
        calls[ci]["epilogue"] = True

    # per-core idx + codes
    idx_cat = np.full((C, tot), -1, np.int16)
    codes = np.full((C, P, m_tot), 255, np.uint8)
    gq_cache = {}
    for call in calls:
        q = call["q"]
        g = call["group"]
        base = (call["col0"] - col_base) * 16
        ncall = call["N"]
        w0 = call["b0"] * P
        if (g, q) not in gq_cache:
            a_tot = sum(int(Lmax[s, q]) for s in groups[g])
            slotid = np.full(a_tot, -1, np.int64)
            idx_full = np.zeros((C, a_tot), np.int16)
            dstp_full = np.full((C, a_tot), 255, np.uint8)
            for s, a in call["A"].items():
                slotid[a:a + int(Lmax[s, q])] = s
                for c in range(C):
                    k = (c * TPC + s) * nsplit + q
                    n = int(L[c, s, q])
                    seg = slice(off[k], off[k] + n)
                    idx_full[c, a:a + n] = srel_s[seg]
                    dstp_full[c, a:a + n] = epart_s[seg]
            gq_cache[(g, q)] = (slotid, idx_full, dstp_full, a_tot)
        slotid, idx_full, dstp_full, a_tot = gq_cache[(g, q)]
        w1 = min(w0 + ncall, a_tot)
        n_real = w1 - w0
        idx_cat[:, base:base + n_real] = idx_full[:, w0:w1]
        for c in range(C):
            dstp = np.full(ncall, 255, np.uint8)
            dstp[:n_real] = dstp_full[c, w0:w1]
            sid = np.full(ncall, -1, np.int64)
            sid[:n_real] = slotid[w0:w1]
            for b, s, m, _, _ in call["descs"]:
                rows = slice(b * P, (b + 1) * P)
                codes[c, :, m - m_base] = np.where(sid[rows] == s,
                                                   dstp[rows], 255)
    return calls, idx_cat, codes, m_tot, tot // 16


def _prep(cfg, x, edge_index):
    import ml_dtypes
    N, C, TPC = cfg.N, cfg.C, cfg.TPC
    src = np.asarray(edge_index[0], dtype=np.int64)
    dst = np.asarray(edge_index[1], dtype=np.int64)
    deg = np.bincount(dst, minlength=N).astype(np.int64) + 1
    dinv = (1.0 / np.sqrt(deg)).astype(np.float32)

    order = np.argsort(-deg, kind="stable")
    i = np.arange(N)
    g_tile = i // P
    lane = g_tile % C
    srow = g_tile // C
    core_of = np.empty(N, np.int64)
    slot_of = np.empty(N, np.int64)
    part_of = np.empty(N, np.int64)
    core_of[order] = np.where(srow % 2 == 0, lane, C - 1 - lane)
    slot_of[order] = srow
    part_of[order] = i % P
    pad_id = (core_of * cfg.NPADL + slot_of * P + part_of).astype(np.int64)

    # self-loops are NOT gathered: they are one dense row-block per slot,
    # added in the epilogues (xs row for L1, cc_in row for L2).  Keeping
    # them in the gather lists put every (core,slot)'s 128 self-edges in
    # one table split, inflating the cross-core Lmax padding by ~9%.
    s_all = src
    d_all = dst

    ecore = core_of[d_all]
    eslot = slot_of[d_all]
    epart = part_of[d_all].astype(np.uint8)

    # layer 1: xs table split into NQ quarters by padded position
    spos = pad_id[s_all]
    equar = spos // cfg.QS
    srel1 = (spos % cfg.QS).astype(np.int16)

    # layer 2: cc table split into NCHUNK chunks by source SLOT range
    scor = core_of[s_all]
    sslt = slot_of[s_all]
    spart = part_of[s_all]
    chunk_of_slot = np.searchsorted(cfg.CH_OFF[1:], sslt, side="right")
    n_k = np.asarray(cfg.CH_SIZES)[chunk_of_slot]
    o_k = cfg.CH_OFF[chunk_of_slot]
    srel2_full = scor * n_k * P + (sslt - o_k) * P + spart
    assert srel2_full.max() < 32768
    srel2 = srel2_full.astype(np.int16)

    calls1, idx1, codes1, m1, cols1 = _schedule(
        cfg, cfg.groups, equar, srel1, ecore, eslot, epart, NQ, 0, 0)
    calls2, idx2, codes2, m2, cols2 = _schedule(
        cfg, cfg.groups, chunk_of_slot, srel2, ecore, eslot, epart,
        NCHUNK, m1, cols1)

    idx_cat = np.concatenate([idx1, idx2], axis=1)
    codes = np.concatenate([codes1, codes2], axis=2)
    cols = cols1 + cols2
    idx_w = idx_cat.reshape(C, cols, 16).transpose(0, 2, 1)
    idx_tabs = np.tile(idx_w, (1, 8, 1)).copy()          # [C,128,cols]
    codes_bf = codes.astype(ml_dtypes.bfloat16)

    dinv_pad = np.zeros(cfg.NPAD, np.float32)
    dinv_pad[pad_id] = dinv
    dinv_tabs = dinv_pad.reshape(C, TPC, P).transpose(0, 2, 1).copy()

    xs_f32 = np.zeros((cfg.NPAD, cfg.F_IN), np.float32)
    xs_f32[pad_id] = np.asarray(x, np.float32) * dinv[:, None]
    xs_pad = xs_f32.astype(ml_dtypes.bfloat16)
    xs_own = xs_f32.reshape(C, cfg.NPADL, cfg.F_IN)

    return dict(calls1=calls1, calls2=calls2, M=m1 + m2, COLS=cols,
                idx_tabs=idx_tabs, codes=codes_bf, dinv_tabs=dinv_tabs,
                xs_pad=xs_pad, xs_own=xs_own, core_of=core_of,
                slot_of=slot_of, part_of=part_of)


def _build(cfg, prep):
    import concourse.bass as bass
    import concourse.bacc as bacc
    import concourse.mybir as mybir
    import concourse.tile as tile
    from concourse.library_config import mlp

    f32 = mybir.dt.float32
    bf16 = mybir.dt.bfloat16
    i16 = mybir.dt.int16
    TPC, QS = cfg.TPC, cfg.QS
    F, FH, FO = cfg.F_IN, cfg.F_HID, cfg.F_OUT
    calls1, calls2, M, COLS = (prep["calls1"], prep["calls2"], prep["M"],
                               prep["COLS"])
    NB = cfg.NBCAP

    nc = bacc.Bacc("TRN2", target_bir_lowering=False, debug=False,
                   enable_asserts=False, num_devices=cfg.C,
                   num_swdge_queues=4,
                   dynamic_dma_scratch_size=cfg.SCRATCH)

    xs_t = nc.dram_tensor("xs", [cfg.NPAD, F], bf16, kind="ExternalInput")
    xso_t = nc.dram_tensor("xso", [cfg.NPADL, F], f32, kind="ExternalInput")
    idx_t = nc.dram_tensor("idx", [P, COLS], i16, kind="ExternalInput")
    codes_t = nc.dram_tensor("codes", [P, M], bf16, kind="ExternalInput")
    dinv_t = nc.dram_tensor("dinv", [P, TPC], f32, kind="ExternalInput")
    w1_t = nc.dram_tensor("w1", [F, FH], f32, kind="ExternalInput")
    b1_t = nc.dram_tensor("b1r", [P, FH], f32, kind="ExternalInput")
    w2_t = nc.dram_tensor("w2", [FH, FO], f32, kind="ExternalInput")
    b2_t = nc.dram_tensor("b2r", [P, FO], f32, kind="ExternalInput")
    ident_t = nc.dram_tensor("identf", [P, P], f32, kind="ExternalInput")
    iota_t = nc.dram_tensor("iota", [P, P], f32, kind="ExternalInput")
    out_t = nc.dram_tensor("out", [cfg.NPADL, FO], f32, kind="ExternalOutput")
    cc_in = nc.dram_tensor("cc_in", [cfg.NPADL, FO], bf16)
    cc_ch = [nc.dram_tensor(f"cc_ch{k}", [cfg.C * cfg.CH_SIZES[k] * P, FO],
                            bf16, addr_space="Shared")
             for k in range(NCHUNK)]

    with tile.TileContext(nc) as tc:
        with (
            tc.tile_pool(name="persist", bufs=1) as pp,
            tc.tile_pool(name="g", bufs=cfg.GBUFS) as gp,
            tc.tile_pool(name="sp", bufs=6) as spool,
            tc.tile_pool(name="ix", bufs=4) as ixp,
            tc.tile_pool(name="ep", bufs=3) as ep,
            tc.tile_pool(name="slp", bufs=8) as slp,
            tc.tile_pool(name="psA", bufs=5, space="PSUM") as psA,
            tc.tile_pool(name="psT", bufs=1, space="PSUM") as psT,
            tc.tile_pool(name="psB", bufs=2, space="PSUM") as psB,
        ):
            nc.gpsimd.load_library(mlp)
            codes_sb = pp.tile([P, M], bf16)
            nc.sync.dma_start(out=codes_sb[:], in_=codes_t[:, :])
            dinv_all = pp.tile([P, TPC], f32)
            nc.sync.dma_start(out=dinv_all[:], in_=dinv_t[:, :])
            ident = pp.tile([P, P], f32)
            nc.sync.dma_start(out=ident[:], in_=ident_t[:, :])
            iota_f = pp.tile([P, P], f32)
            nc.sync.dma_start(out=iota_f[:], in_=iota_t[:, :])
            iota_b = pp.tile([P, P], bf16, tag="iotab")
            nc.vector.tensor_copy(iota_b[:], iota_f[:])
            w_sb = {}
            for nm, wt, fo in (("w1", w1_t, FH), ("w2", w2_t, FO)):
                lst = []
                for k in range(2):
                    w = pp.tile([P, fo], f32, tag=f"{nm}_{k}")
                    nc.sync.dma_start(out=w[:], in_=wt[k * P:(k + 1) * P, :])
                    lst.append(w)
                w_sb[nm] = lst
            b1_sb = pp.tile([P, FH], f32, tag="b1")
            nc.sync.dma_start(out=b1_sb[:], in_=b1_t[:, :])
            b2_sb = pp.tile([P, FO], f32, tag="b2")
            nc.sync.dma_start(out=b2_sb[:], in_=b2_t[:, :])

            for _ in range(cfg.GBUFS):
                gz = gp.tile([P, NB, F], bf16, tag="g")
                nc.vector.memset(gz[:], 0.0)

            def mm_T(psum_out, src_sb, wl):
                nk = src_sb.shape[1] // P
                for k2 in range(nk):
                    psum_tt = psT.tile([P, P], f32, tag="tt")
                    nc.tensor.transpose(psum_tt[:],
                                        src_sb[:, k2 * P:(k2 + 1) * P],
                                        ident[:])
                    sT = ep.tile([P, P], f32, tag="sT")
                    nc.scalar.copy(sT[:], psum_tt[:])
                    nc.tensor.matmul(psum_out[:], lhsT=sT[:], rhs=wl[k2][:],
                                     start=(k2 == 0), stop=(k2 == nk - 1))

            qn = [0]

            def layer(calls, table_of, Fw, first):
                psums = {}
                # batched idx loads: one DMA per (group, q), emitted
                # in-stream right before that (g, q)'s first gather call
                gq_sizes = {}
                for call in calls:
                    k = (call["group"], call["q"])
                    if k not in gq_sizes:
                        gq_sizes[k] = [call["col0"], 0]
                    gq_sizes[k][1] += call["N"] // 16
                cur_ix = [None, None]
                for call in calls:
                    q = call["q"]
                    gq = (call["group"], q)
                    if cur_ix[0] != gq:
                        c0, ctot = gq_sizes[gq]
                        ix = ixp.tile([P, ctot], i16, tag="ix",
                                      padded_shape=[P, (NB * P // 16) * 8])
                        nc.scalar.dma_start(out=ix[:],
                                            in_=idx_t[:, c0:c0 + ctot])
                        cur_ix = [gq, ix]
                    ix = cur_ix[1]
                    o = call["col0"] - gq_sizes[gq][0]
                    nb = call["N"] // P
                    g = gp.tile([P, nb, Fw], bf16, tag="g",
                                padded_shape=[P, NB * (F // Fw), Fw])
                    nc.gpsimd.dma_gather(
                        g[:], table_of(q),
                        ix[:, o:o + call["N"] // 16],
                        call["N"], call["N"], Fw,
                        queue_num=qn[0] % 4)
                    qn[0] += 1
                    descs = call["descs"]
                    nw = len(descs)
                    m0 = descs[0][2]
                    S = spool.tile([P, nw, P], bf16, tag="S",
                                   padded_shape=[P, NB + 6, P])
                    nc.vector.tensor_tensor(
                        out=S[:],
                        in0=codes_sb[:, m0:m0 + nw].unsqueeze(2)
                            .to_broadcast([P, nw, P]),
                        in1=iota_b[:].unsqueeze(1)
                            .to_broadcast([P, nw, P]),
                        op=mybir.AluOpType.is_equal)
                    for j, (b, s, m, st, sp_) in enumerate(descs):
                        if st:
                            psums[s] = psA.tile([P, Fw], f32, tag="agg",
                                                padded_shape=[P, F],
                                                name=f"ps{s}")
                        nc.tensor.matmul(psums[s][:], lhsT=S[:, j, :],
                                         rhs=g[:, b, :], start=st,
                                         stop=sp_)
                    if call["epilogue"]:
                        gslots = cfg.groups[call["group"]]
                        selft = {}
                        for s in gslots:
                            # self-loop term rows, prefetched for the
                            # whole group so DMA latencies overlap
                            if first:
                                xso = slp.tile([P, F], f32, tag="xso")
                                nc.scalar.dma_start(
                                    out=xso[:],
                                    in_=xso_t[s * P:(s + 1) * P, :])
                            else:
                                xso = slp.tile([P, FO], bf16, tag="zsob")
                                nc.scalar.dma_start(
                                    out=xso[:],
                                    in_=cc_in[s * P:(s + 1) * P, :])
                            selft[s] = xso
                        for s in gslots:
                            psum_agg = psums.pop(s, None)
                            if first:
                                xso = selft[s]
                                agg_s = ep.tile([P, F], f32, tag="aggs")
                                if psum_agg is None:
                                    nc.scalar.copy(agg_s[:], xso[:])
                                else:
                                    nc.vector.tensor_add(agg_s[:],
                                                         psum_agg[:],
                                                         xso[:])
                                psum_h = psB.tile([P, FH], f32, tag="h")
                                mm_T(psum_h, agg_s, w_sb["w1"])
                                t1 = ep.tile([P, FH], f32, tag="t1")
                                nc.vector.tensor_scalar_mul(
                                    t1[:], psum_h[:], dinv_all[:, s:s + 1])
                                t2 = ep.tile([P, FH], f32, tag="t2")
                                nc.vector.tensor_add(t2[:], t1[:], b1_sb[:])
                                hs = ep.tile([P, FH], f32, tag="hs")
                                nc.scalar.activation(
                                    hs[:], t2[:],
                                    mybir.ActivationFunctionType.Relu,
                                    scale=dinv_all[:, s:s + 1])
                                psum_o = psB.tile([P, FO], f32, tag="h",
                                                  padded_shape=[P, FH])
                                mm_T(psum_o, hs, w_sb["w2"])
                                os_ = ep.tile([P, FO], bf16, tag="os")
                                nc.vector.tensor_copy(os_[:], psum_o[:])
                                nc.sync.dma_start(
                                    out=cc_in[s * P:(s + 1) * P, :],
                                    in_=os_[:])
                            else:
                                zso_b = selft[s]
                                u0 = ep.tile([P, FO], f32, tag="u0")
                                if psum_agg is None:
                                    nc.scalar.copy(u0[:], zso_b[:])
                                else:
                                    nc.vector.tensor_add(u0[:],
                                                         psum_agg[:],
                                                         zso_b[:])
                                u1 = ep.tile([P, FO], f32, tag="u1")
                                nc.scalar.activation(
                                    u1[:], u0[:],
                                    mybir.ActivationFunctionType.Copy,
                                    scale=dinv_all[:, s:s + 1])
                                u2 = ep.tile([P, FO], f32, tag="u2")
                                nc.vector.tensor_add(u2[:], u1[:], b2_sb[:])
                                nc.sync.dma_start(
                                    out=out_t[s * P:(s + 1) * P, :],
                                    in_=u2[:])
                        if first:
                            for k in range(NCHUNK):
                                if cfg.chunk_last_group[k] == call["group"]:
                                    lo = cfg.CH_OFF[k] * P
                                    hi = cfg.CH_OFF[k + 1] * P
                                    nc.gpsimd.collective_compute(
                                        "AllGather", mybir.AluOpType.bypass,
                                        replica_groups=[list(range(cfg.C))],
                                        ins=[cc_in[lo:hi, :].opt()],
                                        outs=[cc_ch[k].ap().opt()],
                                    )

            mode = os.environ.get("GCN_MODE", "full")
            layer(calls1, lambda q: xs_t[q * QS:(q + 1) * QS, :], F,
                  first=True)
            if mode == "full":
                layer(calls2, lambda k: cc_ch[k][:, :], FO, first=False)

    nc.compile()
    return nc, None


def _run(cfg, nc, prep, W1, b1, W2, b2, trace=False):
    from concourse.bass_utils import run_bass_kernel_spmd
    b1r = np.broadcast_to(np.asarray(b1, np.float32), (P, cfg.F_HID)).copy()
    b2r = np.broadcast_to(np.asarray(b2, np.float32), (P, cfg.F_OUT)).copy()
    iota = np.tile(np.arange(P, dtype=np.float32), (P, 1))
    in_maps = []
    for c in range(cfg.C):
        in_maps.append({
            "xs": prep["xs_pad"],
            "xso": prep["xs_own"][c],
            "idx": prep["idx_tabs"][c],
            "codes": prep["codes"][c],
            "dinv": prep["dinv_tabs"][c],
            "w1": np.asarray(W1, np.float32),
            "b1r": b1r,
            "w2": np.asarray(W2, np.float32),
            "b2r": b2r,
            "identf": np.eye(P, dtype=np.float32),
            "iota": iota,
        })
    res = run_bass_kernel_spmd(nc, in_maps, list(range(cfg.C)), trace=trace)
    outs = np.stack([res.results[c]["out"] for c in range(cfg.C)])
    out_full = np.empty((cfg.N, cfg.F_OUT), np.float32)
    co, so, po = prep["core_of"], prep["slot_of"], prep["part_of"]
    out_full[:] = outs[co, so * P + po]
    return out_full, res


def kernel(x, edge_index, W1, b1, W2, b2):
    cfg = Cfg()
    prep = _prep(cfg, x, edge_index)
    nc, _ = _build(cfg, prep)
    out, _ = _run(cfg, nc, prep, W1, b1, W2, b2,
                  trace=bool(int(os.environ.get("GCN_TRACE", "0"))))
    return out


# revision 17
# speedup vs baseline: 1.3280x; 1.0206x over previous
"""Self-contained Trainium2 Bass kernel for a 2-layer GCN encoder (8 cores).

reference semantics (PyG GCNConv x2):
    out = Ahat @ relu(Ahat @ x @ W1 + b1) @ W2 + b2
    Ahat = D^-1/2 (A + I) D^-1/2,  deg = dst-counts + self-loops.

Strategy (graph/node parallel over 8 NeuronCores):
  * aggregation is linear => reorder matmuls around it:
        layer1: h = relu( dinv * (agg(dinv*x) @ W1) + b1 )
        layer2: out = dinv * agg( (dinv*h) @ W2 ) + b2
    so layer-2 gathers 128-wide rows instead of 256-wide.
  * nodes are degree-sorted into 128-node tiles dealt round-robin to the
    8 cores; every core runs one identical program (SPMD) over its own
    edge data.
  * neighbor rows are fetched with batched InstDMAGatherAnt calls of
    <=1024 rows (HW limit) rotating over the 4 SWDGE queues.
  * gathered rows land at list position (row j -> partition j%128,
    block j//128); a per-call selection matrix (built on DVE from
    precomputed dst-partition codes vs an iota tile) routes rows to
    their destination node partitions in an accumulating PE matmul.
    Pad rows carry code 255 -> zero selection column -> contribute 0.
  * layer-1 dst tiles are processed chunk-major (4 chunks of ~24 slots);
    as each chunk's h'@W2 rows finish, a per-chunk AllGather ships them,
    overlapping the collective with the remaining layer-1 work.  Layer-2
    gathers read the per-chunk replicated tables.
  * per-(tile,split) segment lengths are equalized across cores so the
    instruction stream is core-independent; per-core idx/codes arrays
    carry the data differences.  Trailing pad indices are -1 (skipped by
    HW, no bandwidth); mid-list pads gather row 0 and are masked by
    code 255.
"""

import os
import sys
import numpy as np

for _p in ("/opt/trn_rl_repo",):
    if _p not in sys.path and os.path.isdir(_p):
        sys.path.insert(0, _p)

P = 128
NQ = 4          # xs table quarters (int16 index range)
NCHUNK = 4      # cc chunks (collective pipelining)


class Cfg:
    def __init__(self, N=100000, E=3200000, F_IN=256, F_HID=256, F_OUT=128,
                 C=8, gtiles=4, nbcap=8, scratch=65536, gbufs=14):
        self.N, self.E = N, E
        self.F_IN, self.F_HID, self.F_OUT = F_IN, F_HID, F_OUT
        self.C = C
        self.NBCAP = nbcap          # max 128-row blocks per dma_gather
        self.SCRATCH = scratch      # SWDGE ring bytes per partition
        self.GBUFS = gbufs
        assert nbcap * P <= scratch // 16
        nt = (N + P) // P
        nt = ((nt + C - 1) // C) * C
        self.TPC = nt // C
        self.NT = nt
        self.NPAD = nt * P
        self.NPADL = self.TPC * P
        self.QS = self.NPAD // NQ
        self.GT = gtiles
        assert self.NPAD > self.N and self.NPAD % NQ == 0
        assert self.QS <= 32768
        # cc chunks: split TPC slots into NCHUNK consecutive ranges.
        # The last chunk is small so its AllGather (which gates the tail
        # of every layer-2 group) finishes quickly after layer 1 ends.
        sizes = [30, 30, 28, 10]
        assert sum(sizes) == self.TPC and len(sizes) == NCHUNK
        self.CH_SIZES = sizes                    # slots per chunk
        self.CH_OFF = np.concatenate([[0], np.cumsum(sizes)]).astype(int)
        assert max(s * C * P for s in sizes) <= 32768
        # chunk-major groups of consecutive slots
        self.groups = []
        self.chunk_last_group = {}
        for k in range(NCHUNK):
            lo, hi = self.CH_OFF[k], self.CH_OFF[k + 1]
            s = lo
            while s < hi:
                e = min(s + gtiles, hi)
                self.groups.append(list(range(s, e)))
                s = e
            self.chunk_last_group[k] = len(self.groups) - 1
        self.NG = len(self.groups)


def _schedule(cfg, groups, split_of_edge, srel_of_edge, ecore, eslot, epart,
              nsplit, m_base, col_base):
    """Build a call schedule for one layer.

    split_of_edge: which table-piece each edge's source lives in
    srel_of_edge:  int16 row index within that piece
    Returns (calls, idx_cat, codes, m_tot, tot_cols16).
    """
    C, TPC, NBCAP = cfg.C, cfg.TPC, cfg.NBCAP
    key = (ecore * TPC + eslot) * nsplit + split_of_edge
    ordk = np.argsort(key, kind="stable")
    srel_s = srel_of_edge[ordk]
    epart_s = epart[ordk]
    L = np.bincount(key, minlength=C * TPC * nsplit).reshape(C, TPC, nsplit)
    off = np.zeros(C * TPC * nsplit + 1, np.int64)
    np.cumsum(L.reshape(-1), out=off[1:])
    Lmax = L.max(axis=0)  # [TPC, nsplit]

    calls = []
    first_of = {}
    last_of = {}
    tot = 0
    m_tot = 0
    for g, slots in enumerate(groups):
        for q in range(nsplit):
            A = {}
            a = 0
            for s in slots:
                A[s] = a
                a += int(Lmax[s, q])
            total_len = a
            nb_all = (total_len + P - 1) // P
            b0 = 0
            while b0 < nb_all:
                nb = min(NBCAP, nb_all - b0)
                if nb <= 0:
                    break
                descs = []
                for b in range(b0, b0 + nb):
                    lo, hi = b * P, (b + 1) * P
                    for s in slots:
                        if A[s] < hi and A[s] + int(Lmax[s, q]) > lo:
                            di = (len(calls), len(descs))
                            if s not in first_of:
                                first_of[s] = di
                            last_of[s] = di
                            descs.append([b - b0, s, m_base + m_tot,
                                          False, False])
                            m_tot += 1
                calls.append(dict(q=q, N=nb * P, col0=col_base + tot // 16,
                                  A=A, b0=b0, descs=descs, group=g))
                tot += nb * P
                b0 += nb
    for s, (ci, di) in first_of.items():
        calls[ci]["descs"][di][3] = True
    for s, (ci, di) in last_of.items():
        calls[ci]["descs"][di][4] = True
    glast = {}
    for ci, call in enumerate(calls):
        call["epilogue"] = False
        glast[call["group"]] = ci
    for ci in glast.values():
        calls[ci]["epilogue"] = True

    # per-core idx + codes
    idx_cat = np.full((C, tot), -1, np.int16)
    codes = np.full((C, P, m_tot), 255, np.uint8)
    gq_cache = {}
    for call in calls:
        q = call["q"]
        g = call["group"]
        base = (call["col0"] - col_base) * 16
        ncall = call["N"]
        w0 = call["b0"] * P
        if (g, q) not in gq_cache:
            a_tot = sum(int(Lmax[s, q]) for s in groups[g])
            slotid = np.full(a_tot, -1, np.int64)
            idx_full = np.zeros((C, a_tot), np.int16)
            dstp_full = np.full((C, a_tot), 255, np.uint8)
            for s, a in call["A"].items():
                slotid[a:a + int(Lmax[s, q])] = s
                for c in range(C):
                    k = (c * TPC + s) * nsplit + q
                    n = int(L[c, s, q])
                    seg = slice(off[k], off[k] + n)
                    idx_full[c, a:a + n] = srel_s[seg]
                    dstp_full[c, a:a + n] = epart_s[seg]
            gq_cache[(g, q)] = (slotid, idx_full, dstp_full, a_tot)
        slotid, idx_full, dstp_full, a_tot = gq_cache[(g, q)]
        w1 = min(w0 + ncall, a_tot)
        n_real = w1 - w0
        idx_cat[:, base:base + n_real] = idx_full[:, w0:w1]
        for c in range(C):
            dstp = np.full(ncall, 255, np.uint8)
            dstp[:n_real] = dstp_full[c, w0:w1]
            sid = np.full(ncall, -1, np.int64)
            sid[:n_real] = slotid[w0:w1]
            for b, s, m, _, _ in call["descs"]:
                rows = slice(b * P, (b + 1) * P)
                codes[c, :, m - m_base] = np.where(sid[rows] == s,
                                                   dstp[rows], 255)
    return calls, idx_cat, codes, m_tot, tot // 16


def _prep(cfg, x, edge_index):
    import ml_dtypes
    N, C, TPC = cfg.N, cfg.C, cfg.TPC
    src = np.asarray(edge_index[0], dtype=np.int64)
    dst = np.asarray(edge_index[1], dtype=np.int64)
    deg = np.bincount(dst, minlength=N).astype(np.int64) + 1
    dinv = (1.0 / np.sqrt(deg)).astype(np.float32)

    order = np.argsort(-deg, kind="stable")
    i = np.arange(N)
    g_tile = i // P
    lane = g_tile % C
    srow = g_tile // C
    core_of = np.empty(N, np.int64)
    slot_of = np.empty(N, np.int64)
    part_of = np.empty(N, np.int64)
    core_of[order] = np.where(srow % 2 == 0, lane, C - 1 - lane)
    slot_of[order] = srow
    part_of[order] = i % P
    pad_id = (core_of * cfg.NPADL + slot_of * P + part_of).astype(np.int64)

    # self-loops are NOT gathered: they are one dense row-block per slot,
    # added in the epilogues (xs row for L1, cc_in row for L2).  Keeping
    # them in the gather lists put every (core,slot)'s 128 self-edges in
    # one table split, inflating the cross-core Lmax padding by ~9%.
    s_all = src
    d_all = dst

    ecore = core_of[d_all]
    eslot = slot_of[d_all]
    epart = part_of[d_all].astype(np.uint8)

    # layer 1: xs table split into NQ quarters by padded position
    spos = pad_id[s_all]
    equar = spos // cfg.QS
    srel1 = (spos % cfg.QS).astype(np.int16)

    # layer 2: cc table split into NCHUNK chunks by source SLOT range
    scor = core_of[s_all]
    sslt = slot_of[s_all]
    spart = part_of[s_all]
    chunk_of_slot = np.searchsorted(cfg.CH_OFF[1:], sslt, side="right")
    n_k = np.asarray(cfg.CH_SIZES)[chunk_of_slot]
    o_k = cfg.CH_OFF[chunk_of_slot]
    srel2_full = scor * n_k * P + (sslt - o_k) * P + spart
    assert srel2_full.max() < 32768
    srel2 = srel2_full.astype(np.int16)

    calls1, idx1, codes1, m1, cols1 = _schedule(
        cfg, cfg.groups, equar, srel1, ecore, eslot, epart, NQ, 0, 0)
    calls2, idx2, codes2, m2, cols2 = _schedule(
        cfg, cfg.groups, chunk_of_slot, srel2, ecore, eslot, epart,
        NCHUNK, m1, cols1)

    idx_cat = np.concatenate([idx1, idx2], axis=1)
    codes = np.concatenate([codes1, codes2], axis=2)
    cols = cols1 + cols2
    idx_w = idx_cat.reshape(C, cols, 16).transpose(0, 2, 1)
    idx_tabs = np.tile(idx_w, (1, 8, 1)).copy()          # [C,128,cols]
    codes_bf = codes.astype(ml_dtypes.bfloat16)

    dinv_pad = np.zeros(cfg.NPAD, np.float32)
    dinv_pad[pad_id] = dinv
    dinv_tabs = dinv_pad.reshape(C, TPC, P).transpose(0, 2, 1).copy()

    xs_f32 = np.zeros((cfg.NPAD, cfg.F_IN), np.float32)
    xs_f32[pad_id] = np.asarray(x, np.float32) * dinv[:, None]
    xs_pad = xs_f32.astype(ml_dtypes.bfloat16)
    xs_own = xs_f32.reshape(C, cfg.NPADL, cfg.F_IN)

    return dict(calls1=calls1, calls2=calls2, M=m1 + m2, COLS=cols,
                idx_tabs=idx_tabs, codes=codes_bf, dinv_tabs=dinv_tabs,
                xs_pad=xs_pad, xs_own=xs_own, core_of=core_of,
                slot_of=slot_of, part_of=part_of)


def _build(cfg, prep):
    import concourse.bass as bass
    import concourse.bacc as bacc
    import concourse.mybir as mybir
    import concourse.tile as tile
    from concourse.library_config import mlp

    f32 = mybir.dt.float32
    bf16 = mybir.dt.bfloat16
    i16 = mybir.dt.int16
    TPC, QS = cfg.TPC, cfg.QS
    F, FH, FO = cfg.F_IN, cfg.F_HID, cfg.F_OUT
    calls1, calls2, M, COLS = (prep["calls1"], prep["calls2"], prep["M"],
                               prep["COLS"])
    NB = cfg.NBCAP

    nc = bacc.Bacc("TRN2", target_bir_lowering=False, debug=False,
                   enable_asserts=False, num_devices=cfg.C,
                   num_swdge_queues=4,
                   dynamic_dma_scratch_size=cfg.SCRATCH)

    xs_t = nc.dram_tensor("xs", [cfg.NPAD, F], bf16, kind="ExternalInput")
    xso_t = nc.dram_tensor("xso", [cfg.NPADL, F], f32, kind="ExternalInput")
    idx_t = nc.dram_tensor("idx", [P, COLS], i16, kind="ExternalInput")
    codes_t = nc.dram_tensor("codes", [P, M], bf16, kind="ExternalInput")
    dinv_t = nc.dram_tensor("dinv", [P, TPC], f32, kind="ExternalInput")
    w1_t = nc.dram_tensor("w1", [F, FH], f32, kind="ExternalInput")
    b1_t = nc.dram_tensor("b1r", [P, FH], f32, kind="ExternalInput")
    w2_t = nc.dram_tensor("w2", [FH, FO], f32, kind="ExternalInput")
    b2_t = nc.dram_tensor("b2r", [P, FO], f32, kind="ExternalInput")
    ident_t = nc.dram_tensor("identf", [P, P], f32, kind="ExternalInput")
    iota_t = nc.dram_tensor("iota", [P, P], f32, kind="ExternalInput")
    out_t = nc.dram_tensor("out", [cfg.NPADL, FO], f32, kind="ExternalOutput")
    cc_in = nc.dram_tensor("cc_in", [cfg.NPADL, FO], bf16)
    cc_ch = [nc.dram_tensor(f"cc_ch{k}", [cfg.C * cfg.CH_SIZES[k] * P, FO],
                            bf16, addr_space="Shared")
             for k in range(NCHUNK)]

    with tile.TileContext(nc) as tc:
        with (
            tc.tile_pool(name="persist", bufs=1) as pp,
            tc.tile_pool(name="g", bufs=cfg.GBUFS) as gp,
            tc.tile_pool(name="sp", bufs=8) as spool,
            tc.tile_pool(name="ix", bufs=6) as ixp,
            tc.tile_pool(name="ep", bufs=3) as ep,
            tc.tile_pool(name="slp", bufs=8) as slp,
            tc.tile_pool(name="psA", bufs=5, space="PSUM") as psA,
            tc.tile_pool(name="psT", bufs=1, space="PSUM") as psT,
            tc.tile_pool(name="psB", bufs=2, space="PSUM") as psB,
        ):
            nc.gpsimd.load_library(mlp)
            codes_sb = pp.tile([P, M], bf16)
            nc.sync.dma_start(out=codes_sb[:], in_=codes_t[:, :])
            dinv_all = pp.tile([P, TPC], f32)
            nc.sync.dma_start(out=dinv_all[:], in_=dinv_t[:, :])
            ident = pp.tile([P, P], f32)
            nc.sync.dma_start(out=ident[:], in_=ident_t[:, :])
            iota_f = pp.tile([P, P], f32)
            nc.sync.dma_start(out=iota_f[:], in_=iota_t[:, :])
            iota_b = pp.tile([P, P], bf16, tag="iotab")
            nc.vector.tensor_copy(iota_b[:], iota_f[:])
            w_sb = {}
            for nm, wt, fo in (("w1", w1_t, FH), ("w2", w2_t, FO)):
                lst = []
                for k in range(2):
                    w = pp.tile([P, fo], f32, tag=f"{nm}_{k}")
                    nc.sync.dma_start(out=w[:], in_=wt[k * P:(k + 1) * P, :])
                    lst.append(w)
                w_sb[nm] = lst
            b1_sb = pp.tile([P, FH], f32, tag="b1")
            nc.sync.dma_start(out=b1_sb[:], in_=b1_t[:, :])
            b2_sb = pp.tile([P, FO], f32, tag="b2")
            nc.sync.dma_start(out=b2_sb[:], in_=b2_t[:, :])

            for _ in range(cfg.GBUFS):
                gz = gp.tile([P, NB, F], bf16, tag="g")
                nc.vector.memset(gz[:], 0.0)

            def mm_T(psum_out, src_sb, wl):
                nk = src_sb.shape[1] // P
                for k2 in range(nk):
                    psum_tt = psT.tile([P, P], f32, tag="tt")
                    nc.tensor.transpose(psum_tt[:],
                                        src_sb[:, k2 * P:(k2 + 1) * P],
                                        ident[:])
                    sT = ep.tile([P, P], f32, tag="sT")
                    nc.scalar.copy(sT[:], psum_tt[:])
                    nc.tensor.matmul(psum_out[:], lhsT=sT[:], rhs=wl[k2][:],
                                     start=(k2 == 0), stop=(k2 == nk - 1))

            qn = [0]

            def layer(calls, table_of, Fw, first):
                psums = {}
                # batched idx loads: one DMA per (group, q), emitted
                # in-stream right before that (g, q)'s first gather call
                gq_sizes = {}
                for call in calls:
                    k = (call["group"], call["q"])
                    if k not in gq_sizes:
                        gq_sizes[k] = [call["col0"], 0]
                    gq_sizes[k][1] += call["N"] // 16
                cur_ix = [None, None]
                for call in calls:
                    q = call["q"]
                    gq = (call["group"], q)
                    if cur_ix[0] != gq:
                        c0, ctot = gq_sizes[gq]
                        ix = ixp.tile([P, ctot], i16, tag="ix",
                                      padded_shape=[P, (NB * P // 16) * 8])
                        nc.scalar.dma_start(out=ix[:],
                                            in_=idx_t[:, c0:c0 + ctot])
                        cur_ix = [gq, ix]
                    ix = cur_ix[1]
                    o = call["col0"] - gq_sizes[gq][0]
                    nb = call["N"] // P
                    g = gp.tile([P, nb, Fw], bf16, tag="g",
                                padded_shape=[P, NB * (F // Fw), Fw])
                    nc.gpsimd.dma_gather(
                        g[:], table_of(q),
                        ix[:, o:o + call["N"] // 16],
                        call["N"], call["N"], Fw,
                        queue_num=qn[0] % 4)
                    qn[0] += 1
                    descs = call["descs"]
                    nw = len(descs)
                    m0 = descs[0][2]
                    S = spool.tile([P, nw, P], bf16, tag="S",
                                   padded_shape=[P, NB + 6, P])
                    nc.vector.tensor_tensor(
                        out=S[:],
                        in0=codes_sb[:, m0:m0 + nw].unsqueeze(2)
                            .to_broadcast([P, nw, P]),
                        in1=iota_b[:].unsqueeze(1)
                            .to_broadcast([P, nw, P]),
                        op=mybir.AluOpType.is_equal)
                    for j, (b, s, m, st, sp_) in enumerate(descs):
                        if st:
                            psums[s] = psA.tile([P, Fw], f32, tag="agg",
                                                padded_shape=[P, F],
                                                name=f"ps{s}")
                        nc.tensor.matmul(psums[s][:], lhsT=S[:, j, :],
                                         rhs=g[:, b, :], start=st,
                                         stop=sp_)
                    if call["epilogue"]:
                        gslots = cfg.groups[call["group"]]
                        selft = {}
                        for s in gslots:
                            # self-loop term rows, prefetched for the
                            # whole group so DMA latencies overlap
                            if first:
                                xso = slp.tile([P, F], f32, tag="xso")
                                nc.scalar.dma_start(
                                    out=xso[:],
                                    in_=xso_t[s * P:(s + 1) * P, :])
                            else:
                                xso = slp.tile([P, FO], bf16, tag="zsob")
                                nc.scalar.dma_start(
                                    out=xso[:],
                                    in_=cc_in[s * P:(s + 1) * P, :])
                            selft[s] = xso
                        for s in gslots:
                            psum_agg = psums.pop(s, None)
                            if first:
                                xso = selft[s]
                                agg_s = ep.tile([P, F], f32, tag="aggs")
                                if psum_agg is None:
                                    nc.scalar.copy(agg_s[:], xso[:])
                                else:
                                    nc.vector.tensor_add(agg_s[:],
                                                         psum_agg[:],
                                                         xso[:])
                                psum_h = psB.tile([P, FH], f32, tag="h")
                                mm_T(psum_h, agg_s, w_sb["w1"])
                                t1 = ep.tile([P, FH], f32, tag="t1")
                                nc.vector.tensor_scalar_mul(
                                    t1[:], psum_h[:], dinv_all[:, s:s + 1])
                                t2 = ep.tile([P, FH], f32, tag="t2")
                                nc.vector.tensor_add(t2[:], t1[:], b1_sb[:])
                                hs = ep.tile([P, FH], f32, tag="hs")
                                nc.scalar.activation(
                                    hs[:], t2[:],
                                    mybir.ActivationFunctionType.Relu,
                                    scale=dinv_all[:, s:s + 1])
                                psum_o = psB.tile([P, FO], f32, tag="h",
                                                  padded_shape=[P, FH])
                                mm_T(psum_o, hs, w_sb["w2"])
                                os_ = ep.tile([P, FO], bf16, tag="os")
                                nc.vector.tensor_copy(os_[:], psum_o[:])
                                nc.sync.dma_start(
                                    out=cc_in[s * P:(s + 1) * P, :],
                                    in_=os_[:])
                            else:
                                zso_b = selft[s]
                                u0 = ep.tile([P, FO], f32, tag="u0")
                                if psum_agg is None:
                                    nc.scalar.copy(u0[:], zso_b[:])
                                else:
                                    nc.vector.tensor_add(u0[:],
                                                         psum_agg[:],
                                                         zso_b[:])
                                u1 = ep.tile([P, FO], f32, tag="u1")
                                nc.scalar.activation(
                                    u1[:], u0[:],
                                    mybir.ActivationFunctionType.Copy,
                                    scale=dinv_all[:, s:s + 1])
                                u2 = ep.tile([P, FO], f32, tag="u2")
                                nc.vector.tensor_add(u2[:], u1[:], b2_sb[:])
                                nc.sync.dma_start(
                                    out=out_t[s * P:(s + 1) * P, :],
                                    in_=u2[:])
                        if first:
                            for k in range(NCHUNK):
                                if cfg.chunk_last_group[k] == call["group"]:
                                    lo = cfg.CH_OFF[k] * P
                                    hi = cfg.CH_OFF[k + 1] * P
                                    nc.gpsimd.collective_compute(
                                        "AllGather", mybir.AluOpType.bypass,
                                        replica_groups=[list(range(cfg.C))],
                                        ins=[cc_in[lo:hi, :].opt()],
                                        outs=[cc_ch[k].ap().opt()],
                                    )

            mode = os.environ.get("GCN_MODE", "full")
            layer(calls1, lambda q: xs_t[q * QS:(q + 1) * QS, :], F,
                  first=True)
            if mode == "full":
                layer(calls2, lambda k: cc_ch[k][:, :], FO, first=False)

    nc.compile()
    return nc, None


def _run(cfg, nc, prep, W1, b1, W2, b2, trace=False):
    from concourse.bass_utils import run_bass_kernel_spmd
    b1r = np.broadcast_to(np.asarray(b1, np.float32), (P, cfg.F_HID)).copy()
    b2r = np.broadcast_to(np.asarray(b2, np.float32), (P, cfg.F_OUT)).copy()
    iota = np.tile(np.arange(P, dtype=np.float32), (P, 1))
    in_maps = []
    for c in range(cfg.C):
        in_maps.append({
            "xs": prep["xs_pad"],
            "xso": prep["xs_own"][c],
            "idx": prep["idx_tabs"][c],
            "codes": prep["codes"][c],
            "dinv": prep["dinv_tabs"][c],
            "w1": np.asarray(W1, np.float32),
            "b1r": b1r,
            "w2": np.asarray(W2, np.float32),
            "b2r": b2r,
            "identf": np.eye(P, dtype=np.float32),
            "iota": iota,
        })
    res = run_bass_kernel_spmd(nc, in_maps, list(range(cfg.C)), trace=trace)
    outs = np.stack([res.results[c]["out"] for c in range(cfg.C)])
    out_full = np.empty((cfg.N, cfg.F_OUT), np.float32)
    co, so, po = prep["core_of"], prep["slot_of"], prep["part_of"]
    out_full[:] = outs[co, so * P + po]
    return out_full, res


def kernel(x, edge_index, W1, b1, W2, b2):
    cfg = Cfg()
    prep = _prep(cfg, x, edge_index)
    nc, _ = _build(cfg, prep)
    out, _ = _run(cfg, nc, prep, W1, b1, W2, b2,
                  trace=bool(int(os.environ.get("GCN_TRACE", "0"))))
    return out
